# revision 13
# baseline (speedup 1.0000x reference)
"""Trainium2 Bass kernel for nn_HGNER_22625887716042 (windowed BiLSTM NER head).

Strategy:
- Batch-parallel sharding: core b processes batch row b (B=8 rows, 8 cores).
  Weights replicated; no cross-core communication.
- valid_ids compaction done host-side (pure data movement, like sharding).
- Sliding-window BiLSTM via the "shifted-G" formulation: input projections
  G = Wih @ x.T + bias computed once per chain (3 windows x 2 directions);
  step l of a chain reads a column-shifted slice of G, and sequence-boundary
  masking reduces to updating only a contiguous column range of the h/c state.
- Everything feature-major [features_on_partitions, tokens] so the recurrent
  matmul Whh @ h needs no transposes.
- bf16 matmul operands with fp32 PSUM accumulation; gate nonlinearities on the
  scalar (ACT) engine; state updates on the vector engine (DVE).
"""
import os
import numpy as np
import ml_dtypes

import concourse.bass as bass
import concourse.tile as tile
from concourse import mybir
from concourse.bass_utils import run_bass_kernel_spmd
from concourse.vector_clock import ScopedClock, VectorClock

BF16 = mybir.dt.bfloat16
F32 = mybir.dt.float32
AF = mybir.ActivationFunctionType

B, T, D, NL = 8, 256, 768, 9
H = D // 2                       # 384
G4 = 4 * H                       # 1536
KD = D // 128                    # 6
KH = H // 128                    # 3
M4 = G4 // 128                   # 12
WINDOWS = [3, 5, 7]
CHAINS = [(w, d) for w in WINDOWS for d in (0, 1)]  # (window, dir); dir 0=fwd
NCORES = 8
nbf = ml_dtypes.bfloat16


# ---------------------------------------------------------------------------
# Workaround: walrus CoreV2/V3 CTRL codegen only accepts one sync-wait per
# instruction; TileContext's tail drain accumulates one wait per active
# processor.  Split them across single-wait sync-engine nops.
def _split_drain_and_barrier(self, tick_clock, wait_clock):
    nc = self.nc
    gc = tick_clock.global_clock
    n = len(gc)
    for p in range(n):
        if gc[p] > 0:
            sub = VectorClock([gc[q] if q == p else 0 for q in range(n)])
            nop_inst = nc.sync.nop(nofuse=True)
            wait_clock.add_sem_waits(nop_inst.ins, ScopedClock({None: sub}))
    nc.sync.drain()
    nc.all_engine_barrier()
    assert self.sems is not None
    popped = nc._tile_sem_poison_stack.pop()
    assert popped is self._sem_poison
    nc.clear_and_free_semaphores(list(self.sems.allocated().values()))
    nc.all_engine_barrier()


tile.TileContext._drain_and_barrier = _split_drain_and_barrier

# Same walrus limit applies to regular engine instructions: split any
# instruction carrying more than one sem-wait into single-wait nops on the
# same engine, emitted just before it.
_ORIG_ADD_INST = tile.TileContext._add_instruction
_WSPLIT_N = [0]


def _add_instruction_split_waits(self, inst):
    si = inst.sync_info
    if (
        si is not None
        and si.on_wait
        and len(si.on_wait) > 1
        and inst.engine != mybir.EngineType.Unassigned
    ):
        waits = list(si.on_wait)
        for wv in waits[:-1]:
            _WSPLIT_N[0] += 1
            nop = mybir.InstNoOp(name=f"wsplit_{_WSPLIT_N[0]}", ins=[], outs=[])
            nop.engine = inst.engine
            nop.sync_info = mybir.SyncInfo(on_wait=[wv], on_update=[])
            _ORIG_ADD_INST(self, nop)
        inst.sync_info = mybir.SyncInfo(
            on_wait=waits[-1:], on_update=list(si.on_update)
        )
    _ORIG_ADD_INST(self, inst)


tile.TileContext._add_instruction = _add_instruction_split_waits
# ---------------------------------------------------------------------------


def build_program():
    nc = bass.Bass()
    xT_d = nc.dram_tensor("xT", [D, T], BF16, kind="ExternalInput")
    wih_d = nc.dram_tensor("wih", [6, D, G4], BF16, kind="ExternalInput")
    whh_d = nc.dram_tensor("whh", [6, H, G4], BF16, kind="ExternalInput")
    bias_d = nc.dram_tensor("bias", [128, 6, M4], F32, kind="ExternalInput")
    wlin_d = nc.dram_tensor("wlin", [D, NL], BF16, kind="ExternalInput")
    blin_d = nc.dram_tensor("blin", [NL, 1], F32, kind="ExternalInput")
    ident_d = nc.dram_tensor("ident", [128, 128], BF16, kind="ExternalInput")
    ones_d = nc.dram_tensor("ones", [128, 128], BF16, kind="ExternalInput")
    out_d = nc.dram_tensor("out", [NL, T], F32, kind="ExternalOutput")

    with tile.TileContext(nc) as tc:
        with (
            tc.tile_pool(name="const", bufs=1) as constp,
            tc.tile_pool(name="wihp", bufs=8) as wihp,
            tc.tile_pool(name="gpool", bufs=1) as gpool,
            tc.tile_pool(name="state", bufs=1) as statep,
            tc.tile_pool(name="gates", bufs=3) as gatesp,
            tc.tile_pool(name="tmp", bufs=3) as tmpp,
            tc.tile_pool(name="attn", bufs=2) as attnp,
            tc.tile_pool(name="gps", bufs=5, space="PSUM") as gpsp,
            tc.tile_pool(name="aps", bufs=1, space="PSUM") as apsp,
        ):
            # ---- constants / persistent tensors ----
            xT = constp.tile([128, KD, T], BF16, tag="xT")
            nc.sync.dma_start(xT[:], xT_d.rearrange("(k p) t -> p k t", p=128))
            ident = constp.tile([128, 128], BF16, tag="ident")
            nc.sync.dma_start(ident[:], ident_d[:])
            ones = constp.tile([128, 128], BF16, tag="ones")
            nc.sync.dma_start(ones[:], ones_d[:])
            bias = constp.tile([128, 6, M4], F32, tag="bias")
            nc.sync.dma_start(bias[:], bias_d[:])
            wlin = constp.tile([128, KD, NL], BF16, tag="wlin")
            nc.sync.dma_start(wlin[:], wlin_d.rearrange("(k p) n -> p k n", p=128))
            blin = constp.tile([128, 1], F32, tag="blin")
            nc.sync.dma_start(blin[0:NL, :], blin_d[:])
            whh = constp.tile([128, 18, G4], BF16, tag="whh")
            nc.sync.dma_start(whh[:], whh_d.rearrange("j (k p) n -> p (j k) n", p=128))

            G = [[gpool.tile([128, T], BF16, tag=f"G_{j}_{m}", name=f"G_{j}_{m}") for m in range(M4)]
                 for j in range(6)]
            h_t = [[statep.tile([128, T], BF16, tag=f"h_{j}_{k}", name=f"h_{j}_{k}") for k in range(KH)]
                   for j in range(6)]
            c_t = [[statep.tile([128, T], BF16, tag=f"c_{j}_{k}", name=f"c_{j}_{k}") for k in range(KH)]
                   for j in range(6)]
            for j in range(6):
                for k in range(KH):
                    nc.gpsimd.memset(h_t[j][k][:], 0.0)
                    nc.gpsimd.memset(c_t[j][k][:], 0.0)

            # ---- phase 1: input projections G[j] = WihT[j].T @ xT + bias ----
            for j in range(6):
                wt = []
                for k in range(KD):
                    w = wihp.tile([128, G4], BF16, tag="wih")
                    nc.sync.dma_start(w[:], wih_d[j, 128 * k : 128 * (k + 1), :])
                    wt.append(w)
                for q in range(6):
                    ps = gpsp.tile([128, 2 * T], F32, tag="ps")
                    for sub in range(2):
                        m = 2 * q + sub
                        for k in range(KD):
                            nc.tensor.matmul(
                                ps[:, sub * T : (sub + 1) * T],
                                wt[k][:, 128 * m : 128 * (m + 1)],
                                xT[:, k, :],
                                start=(k == 0),
                                stop=(k == KD - 1),
                                skip_group_check=True,
                            )
                    for sub in range(2):
                        m = 2 * q + sub
                        bap = bias[:, j, m : m + 1]
                        half = ps[:, sub * T : (sub + 1) * T]
                        if m % 2 == 0:
                            nc.scalar.activation(G[j][m][:], half, AF.Identity, bias=bap)
                        else:
                            nc.vector.tensor_scalar_add(G[j][m][:], half, bap)

            # ---- phase 2: recurrent steps (shifted-G windowed LSTM) ----
            for l in range(max(WINDOWS)):
                for j, (w, dirn) in enumerate(CHAINS):
                    if l >= w:
                        continue
                    hw = w // 2
                    off = (l - hw) if dirn == 0 else (hw - l)
                    a = max(0, -off)
                    b2 = min(T, T - off)
                    # gate order is [i,f,o,g] (host-permuted rows): tiles
                    # 0-2=i 3-5=f 6-8=o 9-11=g.  Banks pair (2q, 2q+1); banks
                    # 0-3 pure sigmoid, bank 4 = [o2|g0] mixed, bank 5 tanh.
                    banks = []
                    for q in range(6):
                        ps = gpsp.tile([128, 2 * T], F32, tag="ps")
                        for sub in range(2):
                            m = 2 * q + sub
                            o0 = sub * T
                            nc.tensor.matmul(
                                ps[:, o0 + a : o0 + b2],
                                ident[:],
                                G[j][m][:, a + off : b2 + off],
                                start=True,
                                stop=False,
                                skip_group_check=True,
                            )
                            for k in range(KH):
                                nc.tensor.matmul(
                                    ps[:, o0 : o0 + T],
                                    whh[:, 3 * j + k, 128 * m : 128 * (m + 1)],
                                    h_t[j][k][:],
                                    start=False,
                                    stop=(k == KH - 1),
                                    skip_group_check=True,
                                )
                        gsb = gatesp.tile([128, 2 * T], BF16, tag=f"gate{q}")
                        if q < 4:
                            nc.scalar.activation(gsb[:], ps[:], AF.Sigmoid)
                        elif q == 5:
                            nc.scalar.activation(gsb[:], ps[:], AF.Tanh)
                        else:
                            nc.scalar.activation(gsb[:, 0:T], ps[:, 0:T], AF.Sigmoid)
                            nc.scalar.activation(gsb[:, T : 2 * T], ps[:, T : 2 * T],
                                                 AF.Tanh)
                        banks.append(gsb)

                    def gate(m, lo, hi):
                        return banks[m // 2][:, (m % 2) * T + lo : (m % 2) * T + hi]
                    for r in range(KH):
                        u = tmpp.tile([128, T], BF16, tag=f"u{r}")
                        nc.vector.tensor_mul(u[:, a:b2], gate(r, a, b2),
                                             gate(9 + r, a, b2))
                        t = tmpp.tile([128, T], BF16, tag=f"t{r}")
                        nc.vector.tensor_mul(t[:, a:b2], gate(3 + r, a, b2),
                                             c_t[j][r][:, a:b2])
                        nc.vector.tensor_add(c_t[j][r][:, a:b2], t[:, a:b2],
                                             u[:, a:b2])
                        th = tmpp.tile([128, T], BF16, tag=f"th{r}")
                        nc.scalar.activation(th[:, a:b2], c_t[j][r][:, a:b2], AF.Tanh)
                        nc.vector.tensor_mul(h_t[j][r][:, a:b2], gate(6 + r, a, b2),
                                             th[:, a:b2])

            # ---- phase 3: window attention + linear head ----
            # feature tiles per window: ml[wi] = [h of fwd chain; h of bwd chain]
            ml = [[h_t[2 * wi][k] for k in range(KH)] + [h_t[2 * wi + 1][k] for k in range(KH)]
                  for wi in range(3)]
            dots = apsp.tile([128, T], F32, tag="dot")
            for wi in range(3):
                for k in range(KD):
                    prod = tmpp.tile([128, T], BF16, tag="prod")
                    nc.vector.tensor_mul(prod[:], xT[:, k, :], ml[wi][k][:])
                    nc.tensor.matmul(dots[32 * wi : 32 * wi + 1, :], ones[:, 0:1], prod[:],
                                     start=(k == 0), stop=(k == KD - 1),
                                     skip_group_check=True)
            ds = [attnp.tile([1, T], F32, tag=f"ds{wi}", name=f"ds{wi}")
                  for wi in range(3)]
            for wi in range(3):
                nc.vector.tensor_copy(ds[wi][:], dots[32 * wi : 32 * wi + 1, :])
            mx = attnp.tile([1, T], F32, tag="mx")
            nc.vector.tensor_max(mx[:], ds[0][:], ds[1][:])
            nc.vector.tensor_max(mx[:], mx[:], ds[2][:])
            es = []
            for wi in range(3):
                sd = attnp.tile([1, T], F32, tag=f"sd{wi}")
                nc.vector.tensor_sub(sd[:], ds[wi][:], mx[:])
                e = attnp.tile([1, T], F32, tag=f"e{wi}")
                nc.scalar.activation(e[:], sd[:], AF.Exp, scale=float(1.0 / np.sqrt(D)))
                es.append(e)
            den = attnp.tile([1, T], F32, tag="den")
            nc.vector.tensor_add(den[:], es[0][:], es[1][:])
            nc.vector.tensor_add(den[:], den[:], es[2][:])
            rec = attnp.tile([1, T], F32, tag="rec")
            nc.vector.reciprocal(rec[:], den[:])
            aw = []
            for wi in range(3):
                awt = attnp.tile([1, T], BF16, tag=f"aw{wi}")
                nc.vector.tensor_mul(awt[:], es[wi][:], rec[:])
                aw.append(awt)
            # broadcast attn weights across partitions via K=1 matmul
            Abc = []
            for wi in range(3):
                pb = apsp.tile([128, T], F32, tag="bc")
                nc.tensor.matmul(pb[:], ones[0:1, :], aw[wi][:], start=True, stop=True)
                ab = tmpp.tile([128, T], BF16, tag=f"ab{wi}")
                nc.vector.tensor_copy(ab[:], pb[:])
                Abc.append(ab)
            # xl_k = xT_k + sum_w A_w * ml_w_k ; logits = wlin.T @ xl + blin
            po = apsp.tile([NL, T], F32, tag="po")
            for k in range(KD):
                acc = tmpp.tile([128, T], BF16, tag="acc")
                nc.vector.tensor_mul(acc[:], Abc[0][:], ml[0][k][:])
                t2 = tmpp.tile([128, T], BF16, tag="t2")
                nc.vector.tensor_mul(t2[:], Abc[1][:], ml[1][k][:])
                nc.vector.tensor_add(acc[:], acc[:], t2[:])
                nc.vector.tensor_mul(t2[:], Abc[2][:], ml[2][k][:])
                nc.vector.tensor_add(acc[:], acc[:], t2[:])
                nc.vector.tensor_add(acc[:], acc[:], xT[:, k, :])
                nc.tensor.matmul(po[:], wlin[:, k, :], acc[:],
                                 start=(k == 0), stop=(k == KD - 1))
            osb = attnp.tile([NL, T], F32, tag="osb")
            nc.scalar.activation(osb[:], po[:], AF.Identity, bias=blin[0:NL, :])
            nc.sync.dma_start(out_d[:], osb[:])

    return nc


_NC = None


def _get_program():
    global _NC
    if _NC is None:
        _NC = build_program()
    return _NC


def _host_prep(sequence_output, params, valid_ids):
    seq = np.asarray(sequence_output, np.float32)
    vid = np.asarray(valid_ids)
    # compaction: scatter valid tokens to the front, zeros elsewhere
    vm = vid == 1
    jj = np.where(vm, np.cumsum(vm, axis=1) - 1, T)
    x = np.zeros((B, T + 1, D), np.float32)
    x[np.arange(B)[:, None], jj] = seq
    x = x[:, :T]

    wih = np.empty((6, D, G4), nbf)
    whh = np.empty((6, H, G4), nbf)
    bias = np.empty((128, 6, M4), np.float32)
    for j, (w, dirn) in enumerate(CHAINS):
        wi = WINDOWS.index(w)
        p = params["windows"][wi]["fwd" if dirn == 0 else "bwd"]
        perm = np.concatenate([np.arange(0, 2 * H), np.arange(3 * H, 4 * H),
                               np.arange(2 * H, 3 * H)])
        wih[j] = np.asarray(p["Wih"], np.float32).T[:, perm].astype(nbf)
        whh[j] = np.asarray(p["Whh"], np.float32).T[:, perm].astype(nbf)
        bsum = (np.asarray(p["bih"], np.float32)
                + np.asarray(p["bhh"], np.float32))[perm]
        bias[:, j, :] = bsum.reshape(M4, 128).T
    wlin = np.asarray(params["linear"]["W"], np.float32).T.astype(nbf)
    blin = np.asarray(params["linear"]["b"], np.float32).reshape(NL, 1)
    ident = np.eye(128, dtype=nbf)
    ones = np.ones((128, 128), nbf)

    shared = dict(wih=wih, whh=whh, bias=bias, wlin=wlin, blin=blin,
                  ident=ident, ones=ones)
    in_maps = [dict(shared, xT=np.ascontiguousarray(x[b].T).astype(nbf))
               for b in range(B)]
    return in_maps


def run(sequence_output, params, valid_ids, trace=False):
    nc = _get_program()
    in_maps = _host_prep(sequence_output, params, valid_ids)
    res = run_bass_kernel_spmd(nc, in_maps, list(range(NCORES)), trace=trace)
    out = np.stack([np.asarray(r["out"], np.float32).T for r in res.results])
    return out, res


def kernel(sequence_output, params, valid_ids):
    out, _ = run(sequence_output, params, valid_ids,
                 trace=bool(os.environ.get("KERNEL_TRACE")))
    return out


# revision 14
# speedup vs baseline: 1.0138x; 1.0138x over previous
"""Trainium2 Bass kernel for nn_HGNER_22625887716042 (windowed BiLSTM NER head).

Strategy:
- Batch-parallel sharding: core b processes batch row b (B=8 rows, 8 cores).
  Weights replicated; no cross-core communication.
- valid_ids compaction done host-side (pure data movement, like sharding).
- Sliding-window BiLSTM via the "shifted-G" formulation: input projections
  G = Wih @ x.T + bias computed once per chain (3 windows x 2 directions);
  step l of a chain reads a column-shifted slice of G, and sequence-boundary
  masking reduces to updating only a contiguous column range of the h/c state.
- Everything feature-major [features_on_partitions, tokens] so the recurrent
  matmul Whh @ h needs no transposes.
- bf16 matmul operands with fp32 PSUM accumulation; gate nonlinearities on the
  scalar (ACT) engine; state updates on the vector engine (DVE).
"""
import os
import numpy as np
import ml_dtypes

import concourse.bass as bass
import concourse.tile as tile
from concourse import mybir
from concourse.bass_utils import run_bass_kernel_spmd
from concourse.vector_clock import ScopedClock, VectorClock

BF16 = mybir.dt.bfloat16
F32 = mybir.dt.float32
AF = mybir.ActivationFunctionType

B, T, D, NL = 8, 256, 768, 9
H = D // 2                       # 384
G4 = 4 * H                       # 1536
KD = D // 128                    # 6
KH = H // 128                    # 3
M4 = G4 // 128                   # 12
WINDOWS = [3, 5, 7]
CHAINS = [(w, d) for w in WINDOWS for d in (0, 1)]  # (window, dir); dir 0=fwd
NCORES = 8
nbf = ml_dtypes.bfloat16


# ---------------------------------------------------------------------------
# Workaround: walrus CoreV2/V3 CTRL codegen only accepts one sync-wait per
# instruction; TileContext's tail drain accumulates one wait per active
# processor.  Split them across single-wait sync-engine nops.
def _split_drain_and_barrier(self, tick_clock, wait_clock):
    nc = self.nc
    gc = tick_clock.global_clock
    n = len(gc)
    for p in range(n):
        if gc[p] > 0:
            sub = VectorClock([gc[q] if q == p else 0 for q in range(n)])
            nop_inst = nc.sync.nop(nofuse=True)
            wait_clock.add_sem_waits(nop_inst.ins, ScopedClock({None: sub}))
    nc.sync.drain()
    nc.all_engine_barrier()
    assert self.sems is not None
    popped = nc._tile_sem_poison_stack.pop()
    assert popped is self._sem_poison
    nc.clear_and_free_semaphores(list(self.sems.allocated().values()))
    nc.all_engine_barrier()


tile.TileContext._drain_and_barrier = _split_drain_and_barrier

# Same walrus limit applies to regular engine instructions: split any
# instruction carrying more than one sem-wait into single-wait nops on the
# same engine, emitted just before it.
_ORIG_ADD_INST = tile.TileContext._add_instruction
_WSPLIT_N = [0]


def _add_instruction_split_waits(self, inst):
    si = inst.sync_info
    if (
        si is not None
        and si.on_wait
        and len(si.on_wait) > 1
        and inst.engine != mybir.EngineType.Unassigned
    ):
        waits = list(si.on_wait)
        for wv in waits[:-1]:
            _WSPLIT_N[0] += 1
            nop = mybir.InstNoOp(name=f"wsplit_{_WSPLIT_N[0]}", ins=[], outs=[])
            nop.engine = inst.engine
            nop.sync_info = mybir.SyncInfo(on_wait=[wv], on_update=[])
            _ORIG_ADD_INST(self, nop)
        inst.sync_info = mybir.SyncInfo(
            on_wait=waits[-1:], on_update=list(si.on_update)
        )
    _ORIG_ADD_INST(self, inst)


tile.TileContext._add_instruction = _add_instruction_split_waits
# ---------------------------------------------------------------------------


def build_program():
    nc = bass.Bass()
    xT_d = nc.dram_tensor("xT", [D, T], BF16, kind="ExternalInput")
    wih_d = nc.dram_tensor("wih", [6, D, G4], BF16, kind="ExternalInput")
    whh_d = nc.dram_tensor("whh", [6, H, G4], BF16, kind="ExternalInput")
    bias_d = nc.dram_tensor("bias", [128, 6, M4], F32, kind="ExternalInput")
    wlin_d = nc.dram_tensor("wlin", [D, NL], BF16, kind="ExternalInput")
    blin_d = nc.dram_tensor("blin", [NL, 1], F32, kind="ExternalInput")
    ident_d = nc.dram_tensor("ident", [128, 128], BF16, kind="ExternalInput")
    ones_d = nc.dram_tensor("ones", [128, 128], BF16, kind="ExternalInput")
    out_d = nc.dram_tensor("out", [NL, T], F32, kind="ExternalOutput")

    with tile.TileContext(nc) as tc:
        with (
            tc.tile_pool(name="const", bufs=1) as constp,
            tc.tile_pool(name="wihp", bufs=8) as wihp,
            tc.tile_pool(name="gpool", bufs=1) as gpool,
            tc.tile_pool(name="state", bufs=1) as statep,
            tc.tile_pool(name="gates", bufs=3) as gatesp,
            tc.tile_pool(name="tmp", bufs=3) as tmpp,
            tc.tile_pool(name="attn", bufs=2) as attnp,
            tc.tile_pool(name="gps", bufs=5, space="PSUM") as gpsp,
            tc.tile_pool(name="aps", bufs=1, space="PSUM") as apsp,
        ):
            # ---- constants / persistent tensors ----
            xT = constp.tile([128, KD, T], BF16, tag="xT")
            nc.sync.dma_start(xT[:], xT_d.rearrange("(k p) t -> p k t", p=128))
            ident = constp.tile([128, 128], BF16, tag="ident")
            nc.sync.dma_start(ident[:], ident_d[:])
            ones = constp.tile([128, 128], BF16, tag="ones")
            nc.sync.dma_start(ones[:], ones_d[:])
            bias = constp.tile([128, 6, M4], F32, tag="bias")
            nc.sync.dma_start(bias[:], bias_d[:])
            wlin = constp.tile([128, KD, NL], BF16, tag="wlin")
            nc.sync.dma_start(wlin[:], wlin_d.rearrange("(k p) n -> p k n", p=128))
            blin = constp.tile([128, 1], F32, tag="blin")
            nc.sync.dma_start(blin[0:NL, :], blin_d[:])
            whh = constp.tile([128, 18, G4], BF16, tag="whh")
            nc.sync.dma_start(whh[:], whh_d.rearrange("j (k p) n -> p (j k) n", p=128))

            G = [[gpool.tile([128, T], BF16, tag=f"G_{j}_{m}", name=f"G_{j}_{m}") for m in range(M4)]
                 for j in range(6)]
            h_t = [[statep.tile([128, T], BF16, tag=f"h_{j}_{k}", name=f"h_{j}_{k}") for k in range(KH)]
                   for j in range(6)]
            c_t = [[statep.tile([128, T], BF16, tag=f"c_{j}_{k}", name=f"c_{j}_{k}") for k in range(KH)]
                   for j in range(6)]
            for j in range(6):
                for k in range(KH):
                    nc.gpsimd.memset(h_t[j][k][:], 0.0)
                    nc.gpsimd.memset(c_t[j][k][:], 0.0)

            # ---- phase 1: input projections G[j] = WihT[j].T @ xT + bias ----
            for j in range(6):
                wt = []
                for k in range(KD):
                    w = wihp.tile([128, G4], BF16, tag="wih")
                    nc.sync.dma_start(w[:], wih_d[j, 128 * k : 128 * (k + 1), :])
                    wt.append(w)
                for q in range(6):
                    ps = gpsp.tile([128, 2 * T], F32, tag="ps")
                    for sub in range(2):
                        m = 2 * q + sub
                        for k in range(KD):
                            nc.tensor.matmul(
                                ps[:, sub * T : (sub + 1) * T],
                                wt[k][:, 128 * m : 128 * (m + 1)],
                                xT[:, k, :],
                                start=(k == 0),
                                stop=(k == KD - 1),
                                skip_group_check=True,
                            )
                    for sub in range(2):
                        m = 2 * q + sub
                        bap = bias[:, j, m : m + 1]
                        half = ps[:, sub * T : (sub + 1) * T]
                        if m % 2 == 0:
                            nc.scalar.activation(G[j][m][:], half, AF.Identity, bias=bap)
                        else:
                            nc.vector.tensor_scalar_add(G[j][m][:], half, bap)

            # ---- phase 2: recurrent steps (shifted-G windowed LSTM) ----
            for l in range(max(WINDOWS)):
                for j, (w, dirn) in enumerate(CHAINS):
                    if l >= w:
                        continue
                    hw = w // 2
                    off = (l - hw) if dirn == 0 else (hw - l)
                    a = max(0, -off)
                    b2 = min(T, T - off)
                    # gate order is [i,f,o,g] (host-permuted rows): tiles
                    # 0-2=i 3-5=f 6-8=o 9-11=g.  Banks pair (2q, 2q+1); banks
                    # 0-3 pure sigmoid, bank 4 = [o2|g0] mixed, bank 5 tanh.
                    banks = []
                    for q in range(6):
                        ps = gpsp.tile([128, 2 * T], F32, tag="ps")
                        for sub in range(2):
                            m = 2 * q + sub
                            o0 = sub * T
                            if sub == 0:
                                # G-add on PE via identity matmul
                                nc.tensor.matmul(
                                    ps[:, o0 + a : o0 + b2],
                                    ident[:],
                                    G[j][m][:, a + off : b2 + off],
                                    start=True,
                                    stop=False,
                                    skip_group_check=True,
                                )
                            for k in range(KH):
                                nc.tensor.matmul(
                                    ps[:, o0 : o0 + T],
                                    whh[:, 3 * j + k, 128 * m : 128 * (m + 1)],
                                    h_t[j][k][:],
                                    start=(sub == 1 and k == 0),
                                    stop=(k == KH - 1),
                                    skip_group_check=True,
                                )
                            if sub == 1:
                                # G-add on DVE to offload the PE critical path
                                nc.vector.tensor_add(
                                    ps[:, o0 + a : o0 + b2],
                                    ps[:, o0 + a : o0 + b2],
                                    G[j][m][:, a + off : b2 + off],
                                )
                        gsb = gatesp.tile([128, 2 * T], BF16, tag=f"gate{q}")
                        if q < 4:
                            nc.scalar.activation(gsb[:], ps[:], AF.Sigmoid)
                        elif q == 5:
                            nc.scalar.activation(gsb[:], ps[:], AF.Tanh)
                        else:
                            nc.scalar.activation(gsb[:, 0:T], ps[:, 0:T], AF.Sigmoid)
                            nc.scalar.activation(gsb[:, T : 2 * T], ps[:, T : 2 * T],
                                                 AF.Tanh)
                        banks.append(gsb)

                    def gate(m, lo, hi):
                        return banks[m // 2][:, (m % 2) * T + lo : (m % 2) * T + hi]
                    for r in range(KH):
                        u = tmpp.tile([128, T], BF16, tag=f"u{r}")
                        nc.vector.tensor_mul(u[:, a:b2], gate(r, a, b2),
                                             gate(9 + r, a, b2))
                        t = tmpp.tile([128, T], BF16, tag=f"t{r}")
                        nc.vector.tensor_mul(t[:, a:b2], gate(3 + r, a, b2),
                                             c_t[j][r][:, a:b2])
                        nc.vector.tensor_add(c_t[j][r][:, a:b2], t[:, a:b2],
                                             u[:, a:b2])
                        th = tmpp.tile([128, T], BF16, tag=f"th{r}")
                        nc.scalar.activation(th[:, a:b2], c_t[j][r][:, a:b2], AF.Tanh)
                        nc.vector.tensor_mul(h_t[j][r][:, a:b2], gate(6 + r, a, b2),
                                             th[:, a:b2])

            # ---- phase 3: window attention + linear head ----
            # feature tiles per window: ml[wi] = [h of fwd chain; h of bwd chain]
            ml = [[h_t[2 * wi][k] for k in range(KH)] + [h_t[2 * wi + 1][k] for k in range(KH)]
                  for wi in range(3)]
            dots = apsp.tile([128, T], F32, tag="dot")
            for wi in range(3):
                for k in range(KD):
                    prod = tmpp.tile([128, T], BF16, tag="prod")
                    nc.vector.tensor_mul(prod[:], xT[:, k, :], ml[wi][k][:])
                    nc.tensor.matmul(dots[32 * wi : 32 * wi + 1, :], ones[:, 0:1], prod[:],
                                     start=(k == 0), stop=(k == KD - 1),
                                     skip_group_check=True)
            ds = [attnp.tile([1, T], F32, tag=f"ds{wi}", name=f"ds{wi}")
                  for wi in range(3)]
            for wi in range(3):
                nc.vector.tensor_copy(ds[wi][:], dots[32 * wi : 32 * wi + 1, :])
            mx = attnp.tile([1, T], F32, tag="mx")
            nc.vector.tensor_max(mx[:], ds[0][:], ds[1][:])
            nc.vector.tensor_max(mx[:], mx[:], ds[2][:])
            es = []
            for wi in range(3):
                sd = attnp.tile([1, T], F32, tag=f"sd{wi}")
                nc.vector.tensor_sub(sd[:], ds[wi][:], mx[:])
                e = attnp.tile([1, T], F32, tag=f"e{wi}")
                nc.scalar.activation(e[:], sd[:], AF.Exp, scale=float(1.0 / np.sqrt(D)))
                es.append(e)
            den = attnp.tile([1, T], F32, tag="den")
            nc.vector.tensor_add(den[:], es[0][:], es[1][:])
            nc.vector.tensor_add(den[:], den[:], es[2][:])
            rec = attnp.tile([1, T], F32, tag="rec")
            nc.vector.reciprocal(rec[:], den[:])
            aw = []
            for wi in range(3):
                awt = attnp.tile([1, T], BF16, tag=f"aw{wi}")
                nc.vector.tensor_mul(awt[:], es[wi][:], rec[:])
                aw.append(awt)
            # broadcast attn weights across partitions via K=1 matmul
            Abc = []
            for wi in range(3):
                pb = apsp.tile([128, T], F32, tag="bc")
                nc.tensor.matmul(pb[:], ones[0:1, :], aw[wi][:], start=True, stop=True)
                ab = tmpp.tile([128, T], BF16, tag=f"ab{wi}")
                nc.vector.tensor_copy(ab[:], pb[:])
                Abc.append(ab)
            # xl_k = xT_k + sum_w A_w * ml_w_k ; logits = wlin.T @ xl + blin
            po = apsp.tile([NL, T], F32, tag="po")
            for k in range(KD):
                acc = tmpp.tile([128, T], BF16, tag="acc")
                nc.vector.tensor_mul(acc[:], Abc[0][:], ml[0][k][:])
                t2 = tmpp.tile([128, T], BF16, tag="t2")
                nc.vector.tensor_mul(t2[:], Abc[1][:], ml[1][k][:])
                nc.vector.tensor_add(acc[:], acc[:], t2[:])
                nc.vector.tensor_mul(t2[:], Abc[2][:], ml[2][k][:])
                nc.vector.tensor_add(acc[:], acc[:], t2[:])
                nc.vector.tensor_add(acc[:], acc[:], xT[:, k, :])
                nc.tensor.matmul(po[:], wlin[:, k, :], acc[:],
                                 start=(k == 0), stop=(k == KD - 1))
            osb = attnp.tile([NL, T], F32, tag="osb")
            nc.scalar.activation(osb[:], po[:], AF.Identity, bias=blin[0:NL, :])
            nc.sync.dma_start(out_d[:], osb[:])

    return nc


_NC = None


def _get_program():
    global _NC
    if _NC is None:
        _NC = build_program()
    return _NC


def _host_prep(sequence_output, params, valid_ids):
    seq = np.asarray(sequence_output, np.float32)
    vid = np.asarray(valid_ids)
    # compaction: scatter valid tokens to the front, zeros elsewhere
    vm = vid == 1
    jj = np.where(vm, np.cumsum(vm, axis=1) - 1, T)
    x = np.zeros((B, T + 1, D), np.float32)
    x[np.arange(B)[:, None], jj] = seq
    x = x[:, :T]

    wih = np.empty((6, D, G4), nbf)
    whh = np.empty((6, H, G4), nbf)
    bias = np.empty((128, 6, M4), np.float32)
    for j, (w, dirn) in enumerate(CHAINS):
        wi = WINDOWS.index(w)
        p = params["windows"][wi]["fwd" if dirn == 0 else "bwd"]
        perm = np.concatenate([np.arange(0, 2 * H), np.arange(3 * H, 4 * H),
                               np.arange(2 * H, 3 * H)])
        wih[j] = np.asarray(p["Wih"], np.float32).T[:, perm].astype(nbf)
        whh[j] = np.asarray(p["Whh"], np.float32).T[:, perm].astype(nbf)
        bsum = (np.asarray(p["bih"], np.float32)
                + np.asarray(p["bhh"], np.float32))[perm]
        bias[:, j, :] = bsum.reshape(M4, 128).T
    wlin = np.asarray(params["linear"]["W"], np.float32).T.astype(nbf)
    blin = np.asarray(params["linear"]["b"], np.float32).reshape(NL, 1)
    ident = np.eye(128, dtype=nbf)
    ones = np.ones((128, 128), nbf)

    shared = dict(wih=wih, whh=whh, bias=bias, wlin=wlin, blin=blin,
                  ident=ident, ones=ones)
    in_maps = [dict(shared, xT=np.ascontiguousarray(x[b].T).astype(nbf))
               for b in range(B)]
    return in_maps


def run(sequence_output, params, valid_ids, trace=False):
    nc = _get_program()
    in_maps = _host_prep(sequence_output, params, valid_ids)
    res = run_bass_kernel_spmd(nc, in_maps, list(range(NCORES)), trace=trace)
    out = np.stack([np.asarray(r["out"], np.float32).T for r in res.results])
    return out, res


def kernel(sequence_output, params, valid_ids):
    out, _ = run(sequence_output, params, valid_ids,
                 trace=bool(os.environ.get("KERNEL_TRACE")))
    return out
